# revision 1
# baseline (speedup 1.0000x reference)
"""MinGRU Trainium2 kernel (nn_MinGRU_60421599920446).

Math (per batch row):
    vz[s,h] = x[s,:] @ w_z^T + bz      vh[s,h] = x[s,:] @ w_h^T + bh
    z = sigmoid(vz); h_t = (1-z_t)*h_{t-1} + z_t*vh_t   (scan over s)

Strategy: data-parallel over batch, 1 row per NeuronCore (8 cores).
Per core, work in the transposed domain [H on partitions, S on free] so the
recurrence maps onto the DVE `tensor_tensor_scan` instruction:
    state = a_t * state + b_t,  a = 1-z = sigmoid(-(vz+bz)),  b = z*(vh+bh)

fp32 matmuls on TRN2 run in LOW_HIGH (two-pass) mode with a per-matmul
4-byte LDWEIGHTS, so the matmul domain is bf16: x is cast fp32->bf16 during
the SWDGE DMA load (zero engine cost), PE 128x128 transposes run on bf16,
and the projections use bf16 weights (host-cast) with fp32 PSUM accumulate.
z/a/b and the scan state stay fp32.

Software pipeline per 1024-step s-chunk (output side lags one chunk so the
PE stream never blocks on the serial scan chain):
    gpsimd DMA: x chunk fp32->bf16 (natural [s,d]) ->
    PE transposes -> ACT copies PSUM->SBUF (x^T bf16) ->
    PE bf16 matmuls -> vz/vh PSUM fp32 ->
    ACT: z = Sigmoid(vz+bz), a = Sigmoid(-vz-bz) ->
    DVE: b = (vh + bh) * z   (scalar_tensor_tensor) ->
    DVE: tensor_tensor_scan (carry = last column of prev chunk) ->
    [next iter] PE transposes h -> ACT/DVE copies -> sync DMA out fp32.
"""

import numpy as np
from contextlib import ExitStack

B, S, D, H = 8, 8192, 256, 256
N_CORES = 8
OUT_BF16 = True    # True: scan emits bf16 (faster hT path, ~3e-3 err)

_CACHE = {}


def _build(seq_len, chunk, out_bf16):
    """Build + compile the single-core SPMD Bass program."""
    import concourse.bacc as bacc
    import concourse.tile as tile
    import concourse.mybir as mybir

    dt = mybir.dt
    f32 = dt.float32
    bf16 = dt.bfloat16
    h_dt = bf16 if out_bf16 else f32
    AF = mybir.ActivationFunctionType
    OP = mybir.AluOpType

    assert chunk % 512 == 0 and seq_len % chunk == 0
    nblk = chunk // 128          # 128-row blocks per chunk
    nchunk = seq_len // chunk

    nc = bacc.Bacc("TRN2", target_bir_lowering=False, debug=False)

    x_d = nc.dram_tensor("x", [seq_len, D], f32, kind="ExternalInput").ap()
    wzT_d = nc.dram_tensor("wzT", [D, H], bf16, kind="ExternalInput").ap()
    whT_d = nc.dram_tensor("whT", [D, H], bf16, kind="ExternalInput").ap()
    # packed per-partition columns: [half m][128][h0, bz, -bz, bh]
    cols_d = nc.dram_tensor("cols", [2, 128, 4], f32, kind="ExternalInput").ap()
    idb_d = nc.dram_tensor("identb", [128, 128], bf16, kind="ExternalInput").ap()
    idf_d = nc.dram_tensor("identf", [128, 128], f32, kind="ExternalInput").ap()
    out_d = nc.dram_tensor("out", [seq_len, H], f32, kind="ExternalOutput").ap()

    # chunked views: [chunk-idx, partition(s within block), block, feature]
    x_v = x_d.rearrange("(c t p) d -> c p t d", p=128, t=nblk)
    out_v = out_d.rearrange("(c t p) h -> c p t h", p=128, t=nblk)

    with tile.TileContext(nc) as tc, ExitStack() as ctx:
        const = ctx.enter_context(tc.tile_pool(name="const", bufs=1))
        xin = ctx.enter_context(tc.tile_pool(name="xin", bufs=3))
        xTp = ctx.enter_context(tc.tile_pool(name="xT", bufs=3))
        zp = ctx.enter_context(tc.tile_pool(name="z", bufs=2))
        ap_ = ctx.enter_context(tc.tile_pool(name="a", bufs=2))
        bp = ctx.enter_context(tc.tile_pool(name="b", bufs=2))
        hp = ctx.enter_context(tc.tile_pool(name="h", bufs=3))
        hop = ctx.enter_context(tc.tile_pool(name="ho", bufs=3))
        vzp = ctx.enter_context(tc.tile_pool(name="vz", bufs=2, space="PSUM"))
        vhp = ctx.enter_context(tc.tile_pool(name="vh", bufs=1, space="PSUM"))
        xtrp = ctx.enter_context(tc.tile_pool(name="xtr", bufs=2, space="PSUM"))
        htrp = ctx.enter_context(tc.tile_pool(name="htr", bufs=1, space="PSUM"))

        identb = const.tile([128, 128], bf16, tag="identb")
        nc.sync.dma_start(identb[:], idb_d[:, :])
        ident_h = identb
        if not out_bf16:
            identf = const.tile([128, 128], f32, tag="identf")
            nc.sync.dma_start(identf[:], idf_d[:, :])
            ident_h = identf
        cols = []
        for m in range(2):
            t = const.tile([128, 4], f32, tag=f"cols{m}")
            nc.sync.dma_start(t[:], cols_d[m])
            cols.append(t)
        wzT, whT = [], []
        for k in range(2):
            tz = const.tile([128, H], bf16, tag=f"wz{k}")
            nc.sync.dma_start(tz[:], wzT_d[k * 128:(k + 1) * 128, :])
            wzT.append(tz)
            th = const.tile([128, H], bf16, tag=f"wh{k}")
            nc.sync.dma_start(th[:], whT_d[k * 128:(k + 1) * 128, :])
            whT.append(th)

        h_hist = {}

        def emit_input_side(c, hT_groups):
            """DMA-in, x-transpose, projections, z/a/b, scan for chunk c.
            hT_groups: list of emit-callables for the previous chunk's
            h-transpose groups, interleaved with this chunk's x-transpose
            groups to fill PE ping-pong bubbles."""
            xn = xin.tile([128, nblk * D], bf16, tag="xn", name="xn")
            nc.gpsimd.dma_start(          # SWDGE: casts fp32 -> bf16
                xn[:].rearrange("p (t d) -> p t d", d=D), x_v[c])

            xT = [xTp.tile([128, chunk], bf16, tag=f"xt{k}", name=f"xt{k}")
                  for k in range(2)]
            gi = 0
            for k in range(2):
                for g in range(chunk // 512):
                    pt = xtrp.tile([128, 512], bf16, tag="xtr", name="ptx")
                    for j in range(4):
                        t = g * 4 + j
                        nc.tensor.transpose(
                            pt[:, j * 128:(j + 1) * 128],
                            xn[:, t * D + k * 128: t * D + (k + 1) * 128],
                            identb[:],
                        )
                    nc.scalar.copy(xT[k][:, g * 512:(g + 1) * 512], pt[:])
                    if gi < len(hT_groups):
                        hT_groups[gi]()
                        gi += 1
            for f in hT_groups[gi:]:
                f()

            vz = [vzp.tile([128, chunk], f32, tag="vz", name=f"vz{m}")
                  for m in range(2)]
            for m in range(2):
                for k in range(2):
                    for s2 in range(chunk // 512):
                        nc.tensor.matmul(
                            vz[m][:, s2 * 512:(s2 + 1) * 512],
                            wzT[k][:, m * 128:(m + 1) * 128],
                            xT[k][:, s2 * 512:(s2 + 1) * 512],
                            start=(k == 0), stop=(k == 1),
                        )
            z = [zp.tile([128, chunk], f32, tag=f"z{m}", name=f"z{m}")
                 for m in range(2)]
            a = [ap_.tile([128, chunk], f32, tag=f"a{m}", name=f"a{m}")
                 for m in range(2)]
            for m in range(2):
                nc.scalar.activation(z[m][:], vz[m][:], AF.Sigmoid,
                                     bias=cols[m][:, 1:2], scale=1.0)
                nc.scalar.activation(a[m][:], vz[m][:], AF.Sigmoid,
                                     bias=cols[m][:, 2:3], scale=-1.0)

            b = [bp.tile([128, chunk], f32, tag=f"b{m}", name=f"b{m}")
                 for m in range(2)]
            for m in range(2):
                for s2 in range(chunk // 512):
                    vht = vhp.tile([128, 512], f32, tag="vh", name="vht")
                    for k in range(2):
                        nc.tensor.matmul(
                            vht[:],
                            whT[k][:, m * 128:(m + 1) * 128],
                            xT[k][:, s2 * 512:(s2 + 1) * 512],
                            start=(k == 0), stop=(k == 1),
                        )
                    nc.vector.scalar_tensor_tensor(
                        b[m][:, s2 * 512:(s2 + 1) * 512],
                        vht[:], cols[m][:, 3:4],
                        z[m][:, s2 * 512:(s2 + 1) * 512],
                        op0=OP.add, op1=OP.mult,
                    )

            h = [hp.tile([128, chunk], h_dt, tag=f"h{m}", name=f"h{m}")
                 for m in range(2)]
            for m in range(2):
                init = (cols[m][:, 0:1] if c == 0
                        else h_hist[c - 1][m][:, chunk - 1:chunk])
                nc.vector.tensor_tensor_scan(
                    h[m][:], a[m][:], b[m][:], init,
                    op0=OP.mult, op1=OP.add,
                )
            h_hist[c] = h

        def make_output_groups(c):
            """h-transpose back to natural [s, h] + store for chunk c,
            as per-group emit callables."""
            h = h_hist[c]
            ho = hop.tile([128, nblk * H], f32, tag="ho", name="ho")
            ngroups = chunk // 256

            def make(g):
                def emit():
                    pt = htrp.tile([128, 512], h_dt, tag="htr", name="pth")
                    for j in range(2):
                        t = g * 2 + j
                        for m in range(2):
                            nc.tensor.transpose(
                                pt[:, j * 256 + m * 128: j * 256 + (m + 1) * 128],
                                h[m][:, t * 128:(t + 1) * 128],
                                ident_h[:],
                            )
                    # split the PSUM->SBUF copies between ACT and DVE
                    if g % 2 == 0:
                        nc.scalar.copy(ho[:, g * 512:(g + 1) * 512], pt[:])
                    else:
                        nc.vector.tensor_copy(ho[:, g * 512:(g + 1) * 512], pt[:])
                    if g == ngroups - 1:
                        nc.sync.dma_start(
                            out_v[c], ho[:].rearrange("p (t h) -> p t h", h=H))
                return emit
            return [make(g) for g in range(ngroups)]

        for c in range(nchunk + 1):
            groups = make_output_groups(c - 1) if c >= 1 else []
            if c < nchunk:
                emit_input_side(c, groups)
            else:
                for f in groups:
                    f()

    nc.compile()
    return nc


def _get(seq_len, chunk, out_bf16=OUT_BF16):
    key = (seq_len, chunk, out_bf16)
    if key not in _CACHE:
        _CACHE[key] = _build(seq_len, chunk, out_bf16)
    return _CACHE[key]


def _make_in_maps(x, h0, w_h_w, w_h_b, w_z_w, w_z_b, n_cores=N_CORES):
    import ml_dtypes
    bf16 = ml_dtypes.bfloat16
    wzT = np.ascontiguousarray(np.asarray(w_z_w, np.float32).T.astype(bf16))
    whT = np.ascontiguousarray(np.asarray(w_h_w, np.float32).T.astype(bf16))
    bz = np.asarray(w_z_b, np.float32).reshape(2, 128)
    bh = np.asarray(w_h_b, np.float32).reshape(2, 128)
    identf = np.eye(128, dtype=np.float32)
    identb = identf.astype(bf16)
    in_maps = []
    for i in range(n_cores):
        h0c = np.asarray(h0[i, 0], np.float32).reshape(2, 128)
        cols = np.stack([h0c, bz, -bz, bh], axis=-1)  # [2,128,4]
        in_maps.append({
            "x": np.ascontiguousarray(np.asarray(x[i], np.float32)),
            "wzT": wzT, "whT": whT,
            "cols": np.ascontiguousarray(cols),
            "identb": identb, "identf": identf,
        })
    return in_maps


def kernel(x, h0, w_h_w, w_h_b, w_z_w, w_z_b):
    from concourse.bass_utils import run_bass_kernel_spmd

    nc = _get(S, 1024)
    in_maps = _make_in_maps(x, h0, w_h_w, w_h_b, w_z_w, w_z_b)
    res = run_bass_kernel_spmd(nc, in_maps, list(range(N_CORES)))
    out = np.stack([res.results[i]["out"] for i in range(N_CORES)], axis=0)
    return out.astype(np.float32)



# revision 3
# speedup vs baseline: 1.2390x; 1.2390x over previous
"""MinGRU Trainium2 kernel (nn_MinGRU_60421599920446), v2.

Math (per batch row):
    vz[s,h] = x[s,:] @ w_z^T + bz      vh[s,h] = x[s,:] @ w_h^T + bh
    z = sigmoid(vz); h_t = (1-z_t)*h_{t-1} + z_t*vh_t   (scan over s)

Strategy: data-parallel over batch, 1 row per NeuronCore (8 cores).
All tensors live in the transposed domain [channel on partitions, S on free]
end to end: the host pre-transposes x to xT (bf16) and post-transposes the
returned hT, so the kernel does NO on-chip transposes (the v1 baseline spent
>half its PE time on them) and moves only bf16 over HBM (8 MB/core).

Per-engine assignment (measured rates, ns per 128-col):
    PE    : 128 bf16 matmuls N=512 (vz, vh)                  ~28 us
    ACT   : z = Sigmoid(vz+bz), hbar = Identity(vh+bh)       ~37 us
    DVE   : tensor_tensor_scan (2 cyc/col, dtype-independent) ~36 us
            + a slice of b = z*hbar (bf16 tt, 2x_1p)
    GPSIMD: a = 1-z (tensor_scalar) + rest of b = z*hbar     ~40 us
    DMA   : xT in (4 MB) + hT out (4 MB), HWDGE              ~22 us

The scan is chained across 2048-col pairs via the previous pair's last
column; m-halves (two 128-channel groups) are processed in two outer passes
so PSUM holds vz/vh [128,1024] double-buffered (8 banks exactly).
"""

import numpy as np
from contextlib import ExitStack

B, S, D, H = 8, 8192, 256, 256
N_CORES = 8
CHUNK = 1024          # PSUM tile width (2 banks) and ACT instruction width
DVE_B = 384           # columns of each b-chunk computed on DVE (rest GPSIMD)

_CACHE = {}


def _build(seq_len, chunk, dve_b=DVE_B):
    """Build + compile the single-core SPMD Bass program."""
    import concourse.bacc as bacc
    import concourse.tile as tile
    import concourse.mybir as mybir

    dt = mybir.dt
    f32 = dt.float32
    bf16 = dt.bfloat16
    AF = mybir.ActivationFunctionType
    OP = mybir.AluOpType

    assert chunk % 512 == 0 and seq_len % (2 * chunk) == 0
    nchunk = seq_len // chunk
    pair = 2 * chunk

    nc = bacc.Bacc("TRN2", target_bir_lowering=False, debug=False)

    xT_d = nc.dram_tensor("xT", [2, 128, seq_len], bf16, kind="ExternalInput").ap()
    wz_d = nc.dram_tensor("wz", [2, 128, H], bf16, kind="ExternalInput").ap()
    wh_d = nc.dram_tensor("wh", [2, 128, H], bf16, kind="ExternalInput").ap()
    # packed per-partition columns: [half m][128][h0, bz, bh]
    cols_d = nc.dram_tensor("cols", [2, 128, 3], f32, kind="ExternalInput").ap()
    out_d = nc.dram_tensor("out", [2, 128, seq_len], bf16, kind="ExternalOutput").ap()

    with tile.TileContext(nc) as tc, ExitStack() as ctx:
        const = ctx.enter_context(tc.tile_pool(name="const", bufs=1))
        xin = ctx.enter_context(tc.tile_pool(name="xin", bufs=1))
        zp = ctx.enter_context(tc.tile_pool(name="z", bufs=3))
        hbp = ctx.enter_context(tc.tile_pool(name="hb", bufs=3))
        ap_ = ctx.enter_context(tc.tile_pool(name="a", bufs=2))
        bp = ctx.enter_context(tc.tile_pool(name="b", bufs=2))
        hp = ctx.enter_context(tc.tile_pool(name="h", bufs=3))
        vzp = ctx.enter_context(tc.tile_pool(name="vz", bufs=2, space="PSUM"))
        vhp = ctx.enter_context(tc.tile_pool(name="vh", bufs=2, space="PSUM"))

        cols = []
        for m in range(2):
            t = const.tile([128, 3], f32, tag=f"cols{m}")
            nc.sync.dma_start(t[:], cols_d[m])
            cols.append(t)
        wz, wh = [], []
        for k in range(2):
            tz = const.tile([128, H], bf16, tag=f"wz{k}")
            nc.sync.dma_start(tz[:], wz_d[k])
            wz.append(tz)
            th = const.tile([128, H], bf16, tag=f"wh{k}")
            nc.sync.dma_start(th[:], wh_d[k])
            wh.append(th)

        # resident xT chunks: one tagged tile per (k, c)
        xc = [[xin.tile([128, chunk], bf16, tag=f"x{k}_{c}", name=f"x{k}_{c}")
               for c in range(nchunk)] for k in range(2)]

        for m in range(2):
            h_prev = None
            a_pair = b_pair = None
            for c in range(nchunk):
                if m == 0:
                    for k in range(2):
                        nc.sync.dma_start(
                            xc[k][c][:], xT_d[k, :, c * chunk:(c + 1) * chunk])

                vz = vzp.tile([128, chunk], f32, tag="vz", name=f"vz{m}_{c}")
                vh = vhp.tile([128, chunk], f32, tag="vh", name=f"vh{m}_{c}")
                for k in range(2):
                    for s2 in range(chunk // 512):
                        nc.tensor.matmul(
                            vz[:, s2 * 512:(s2 + 1) * 512],
                            wz[k][:, m * 128:(m + 1) * 128],
                            xc[k][c][:, s2 * 512:(s2 + 1) * 512],
                            start=(k == 0), stop=(k == 1))
                for k in range(2):
                    for s2 in range(chunk // 512):
                        nc.tensor.matmul(
                            vh[:, s2 * 512:(s2 + 1) * 512],
                            wh[k][:, m * 128:(m + 1) * 128],
                            xc[k][c][:, s2 * 512:(s2 + 1) * 512],
                            start=(k == 0), stop=(k == 1))

                z = zp.tile([128, chunk], bf16, tag="z", name=f"z{m}_{c}")
                nc.scalar.activation(z[:], vz[:], AF.Sigmoid,
                                     bias=cols[m][:, 1:2], scale=1.0)
                hb = hbp.tile([128, chunk], bf16, tag="hb", name=f"hb{m}_{c}")
                nc.scalar.activation(hb[:], vh[:], AF.Identity,
                                     bias=cols[m][:, 2:3], scale=1.0)

                if c % 2 == 0:
                    a_pair = ap_.tile([128, pair], bf16, tag="a",
                                      name=f"a{m}_{c // 2}")
                    b_pair = bp.tile([128, pair], bf16, tag="b",
                                     name=f"b{m}_{c // 2}")
                off = (c % 2) * chunk
                nc.gpsimd.tensor_scalar(a_pair[:, off:off + chunk], z[:],
                                        -1.0, 1.0, op0=OP.mult, op1=OP.add)
                nc.vector.tensor_tensor(
                    b_pair[:, off:off + dve_b],
                    z[:, :dve_b], hb[:, :dve_b], op=OP.mult)
                nc.gpsimd.tensor_tensor(
                    b_pair[:, off + dve_b:off + chunk],
                    z[:, dve_b:], hb[:, dve_b:], op=OP.mult)

                if c % 2 == 1:
                    p = c // 2
                    h = hp.tile([128, pair], bf16, tag="h", name=f"h{m}_{p}")
                    init = (cols[m][:, 0:1] if p == 0
                            else h_prev[:, pair - 1:pair])
                    nc.vector.tensor_tensor_scan(
                        h[:], a_pair[:], b_pair[:], init,
                        op0=OP.mult, op1=OP.add)
                    nc.sync.dma_start(
                        out_d[m, :, p * pair:(p + 1) * pair], h[:])
                    h_prev = h

    nc.compile()
    return nc


def _get(seq_len, chunk, dve_b=DVE_B):
    key = (seq_len, chunk, dve_b)
    if key not in _CACHE:
        _CACHE[key] = _build(seq_len, chunk, dve_b)
    return _CACHE[key]


def _make_in_maps(x, h0, w_h_w, w_h_b, w_z_w, w_z_b, n_cores=N_CORES):
    import ml_dtypes
    bf16 = ml_dtypes.bfloat16
    wzT = np.asarray(w_z_w, np.float32).T.astype(bf16).reshape(2, 128, H)
    whT = np.asarray(w_h_w, np.float32).T.astype(bf16).reshape(2, 128, H)
    bz = np.asarray(w_z_b, np.float32).reshape(2, 128)
    bh = np.asarray(w_h_b, np.float32).reshape(2, 128)
    in_maps = []
    for i in range(n_cores):
        h0c = np.asarray(h0[i, 0], np.float32).reshape(2, 128)
        cols = np.stack([h0c, bz, bh], axis=-1)  # [2,128,3]
        xT = np.ascontiguousarray(np.asarray(x[i], np.float32).T).astype(bf16)
        in_maps.append({
            "xT": np.ascontiguousarray(xT.reshape(2, 128, -1)),
            "wz": np.ascontiguousarray(wzT),
            "wh": np.ascontiguousarray(whT),
            "cols": np.ascontiguousarray(cols),
        })
    return in_maps


def kernel(x, h0, w_h_w, w_h_b, w_z_w, w_z_b):
    from concourse.bass_utils import run_bass_kernel_spmd

    nc = _get(S, CHUNK)
    in_maps = _make_in_maps(x, h0, w_h_w, w_h_b, w_z_w, w_z_b)
    res = run_bass_kernel_spmd(nc, in_maps, list(range(N_CORES)))
    out = np.empty((N_CORES, S, H), dtype=np.float32)
    for i in range(N_CORES):
        hT = np.asarray(res.results[i]["out"]).reshape(H, S)
        out[i] = hT.astype(np.float32).T
    return out


# revision 4
# speedup vs baseline: 1.4416x; 1.1635x over previous
"""MinGRU Trainium2 kernel (nn_MinGRU_60421599920446), v2.

Math (per batch row):
    vz[s,h] = x[s,:] @ w_z^T + bz      vh[s,h] = x[s,:] @ w_h^T + bh
    z = sigmoid(vz); h_t = (1-z_t)*h_{t-1} + z_t*vh_t   (scan over s)

Strategy: data-parallel over batch, 1 row per NeuronCore (8 cores).
All tensors live in the transposed domain [channel on partitions, S on free]
end to end: the host pre-transposes x to xT (bf16) and post-transposes the
returned hT, so the kernel does NO on-chip transposes (the v1 baseline spent
>half its PE time on them) and moves only bf16 over HBM (8 MB/core).

Per-engine assignment (measured rates, ns per 128-col):
    PE    : 128 bf16 matmuls N=512 (vz, vh)                  ~28 us
    ACT   : z = Sigmoid(vz+bz), hbar = Identity(vh+bh)       ~37 us
    DVE   : tensor_tensor_scan (2 cyc/col, dtype-independent) ~36 us
            + a slice of b = z*hbar (bf16 tt, 2x_1p)
    GPSIMD: a = 1-z (tensor_scalar) + rest of b = z*hbar     ~40 us
    DMA   : xT in (4 MB) + hT out (4 MB), HWDGE              ~22 us

The scan is chained across 2048-col pairs via the previous pair's last
column; m-halves (two 128-channel groups) are processed in two outer passes
so PSUM holds vz/vh [128,1024] double-buffered (8 banks exactly).
"""

import numpy as np
from contextlib import ExitStack

B, S, D, H = 8, 8192, 256, 256
N_CORES = 8
CHUNK = 1024          # PSUM tile width (2 banks) and ACT instruction width
DVE_B = 384           # columns of each b-chunk computed on DVE (rest GPSIMD)

_CACHE = {}


def _build(seq_len, chunk, dve_b=DVE_B):
    """Build + compile the single-core SPMD Bass program."""
    import concourse.bacc as bacc
    import concourse.tile as tile
    import concourse.mybir as mybir

    dt = mybir.dt
    f32 = dt.float32
    bf16 = dt.bfloat16
    AF = mybir.ActivationFunctionType
    OP = mybir.AluOpType

    assert chunk % 512 == 0 and seq_len % (2 * chunk) == 0
    nchunk = seq_len // chunk
    pair = 2 * chunk

    nc = bacc.Bacc("TRN2", target_bir_lowering=False, debug=False)

    xT_d = nc.dram_tensor("xT", [2, 128, seq_len], bf16, kind="ExternalInput").ap()
    wz_d = nc.dram_tensor("wz", [2, 128, H], bf16, kind="ExternalInput").ap()
    wh_d = nc.dram_tensor("wh", [2, 128, H], bf16, kind="ExternalInput").ap()
    # packed per-partition columns: [half m][128][h0, bz, bh]
    cols_d = nc.dram_tensor("cols", [2, 128, 3], f32, kind="ExternalInput").ap()
    out_d = nc.dram_tensor("out", [2, 128, seq_len], bf16, kind="ExternalOutput").ap()

    with tile.TileContext(nc) as tc, ExitStack() as ctx:
        const = ctx.enter_context(tc.tile_pool(name="const", bufs=1))
        xin = ctx.enter_context(tc.tile_pool(name="xin", bufs=1))
        zp = ctx.enter_context(tc.tile_pool(name="z", bufs=3))
        hbp = ctx.enter_context(tc.tile_pool(name="hb", bufs=3))
        ap_ = ctx.enter_context(tc.tile_pool(name="a", bufs=2))
        bp = ctx.enter_context(tc.tile_pool(name="b", bufs=2))
        hp = ctx.enter_context(tc.tile_pool(name="h", bufs=3))
        vzp = ctx.enter_context(tc.tile_pool(name="vz", bufs=2, space="PSUM"))
        vhp = ctx.enter_context(tc.tile_pool(name="vh", bufs=2, space="PSUM"))

        cols = []
        for m in range(2):
            t = const.tile([128, 3], f32, tag=f"cols{m}")
            nc.sync.dma_start(t[:], cols_d[m])
            cols.append(t)
        wz, wh = [], []
        for k in range(2):
            tz = const.tile([128, H], bf16, tag=f"wz{k}")
            nc.sync.dma_start(tz[:], wz_d[k])
            wz.append(tz)
            th = const.tile([128, H], bf16, tag=f"wh{k}")
            nc.sync.dma_start(th[:], wh_d[k])
            wh.append(th)

        # resident xT chunks: one tagged tile per (k, c)
        xc = [[xin.tile([128, chunk], bf16, tag=f"x{k}_{c}", name=f"x{k}_{c}")
               for c in range(nchunk)] for k in range(2)]

        for m in range(2):
            h_prev = None
            a_pair = b_pair = None
            for c in range(nchunk):
                if m == 0:
                    for k in range(2):
                        nc.sync.dma_start(
                            xc[k][c][:], xT_d[k, :, c * chunk:(c + 1) * chunk])

                vz = vzp.tile([128, chunk], f32, tag="vz", name=f"vz{m}_{c}")
                vh = vhp.tile([128, chunk], f32, tag="vh", name=f"vh{m}_{c}")
                for k in range(2):
                    for s2 in range(chunk // 512):
                        nc.tensor.matmul(
                            vz[:, s2 * 512:(s2 + 1) * 512],
                            wz[k][:, m * 128:(m + 1) * 128],
                            xc[k][c][:, s2 * 512:(s2 + 1) * 512],
                            start=(k == 0), stop=(k == 1))
                for k in range(2):
                    for s2 in range(chunk // 512):
                        nc.tensor.matmul(
                            vh[:, s2 * 512:(s2 + 1) * 512],
                            wh[k][:, m * 128:(m + 1) * 128],
                            xc[k][c][:, s2 * 512:(s2 + 1) * 512],
                            start=(k == 0), stop=(k == 1))

                z = zp.tile([128, chunk], bf16, tag="z", name=f"z{m}_{c}")
                nc.scalar.activation(z[:], vz[:], AF.Sigmoid,
                                     bias=cols[m][:, 1:2], scale=1.0)
                hb = hbp.tile([128, chunk], bf16, tag="hb", name=f"hb{m}_{c}")
                nc.scalar.activation(hb[:], vh[:], AF.Identity,
                                     bias=cols[m][:, 2:3], scale=1.0)

                if c % 2 == 0:
                    a_pair = ap_.tile([128, pair], bf16, tag="a",
                                      name=f"a{m}_{c // 2}")
                    b_pair = bp.tile([128, pair], bf16, tag="b",
                                     name=f"b{m}_{c // 2}")
                off = (c % 2) * chunk
                nc.vector.tensor_tensor(
                    b_pair[:, off:off + chunk], z[:], hb[:], op=OP.mult)
                nc.gpsimd.tensor_scalar(a_pair[:, off:off + chunk], z[:],
                                        -1.0, 1.0, op0=OP.mult, op1=OP.add)

                if c % 2 == 1:
                    p = c // 2
                    h = hp.tile([128, pair], bf16, tag="h", name=f"h{m}_{p}")
                    init = (cols[m][:, 0:1] if p == 0
                            else h_prev[:, pair - 1:pair])
                    nc.vector.tensor_tensor_scan(
                        h[:], a_pair[:], b_pair[:], init,
                        op0=OP.mult, op1=OP.add)
                    nc.sync.dma_start(
                        out_d[m, :, p * pair:(p + 1) * pair], h[:])
                    h_prev = h

    nc.compile()
    return nc


def _get(seq_len, chunk, dve_b=DVE_B):
    key = (seq_len, chunk, dve_b)
    if key not in _CACHE:
        _CACHE[key] = _build(seq_len, chunk, dve_b)
    return _CACHE[key]


def _make_in_maps(x, h0, w_h_w, w_h_b, w_z_w, w_z_b, n_cores=N_CORES):
    import ml_dtypes
    bf16 = ml_dtypes.bfloat16
    wzT = np.asarray(w_z_w, np.float32).T.astype(bf16).reshape(2, 128, H)
    whT = np.asarray(w_h_w, np.float32).T.astype(bf16).reshape(2, 128, H)
    bz = np.asarray(w_z_b, np.float32).reshape(2, 128)
    bh = np.asarray(w_h_b, np.float32).reshape(2, 128)
    in_maps = []
    for i in range(n_cores):
        h0c = np.asarray(h0[i, 0], np.float32).reshape(2, 128)
        cols = np.stack([h0c, bz, bh], axis=-1)  # [2,128,3]
        xT = np.ascontiguousarray(np.asarray(x[i], np.float32).T).astype(bf16)
        in_maps.append({
            "xT": np.ascontiguousarray(xT.reshape(2, 128, -1)),
            "wz": np.ascontiguousarray(wzT),
            "wh": np.ascontiguousarray(whT),
            "cols": np.ascontiguousarray(cols),
        })
    return in_maps


def kernel(x, h0, w_h_w, w_h_b, w_z_w, w_z_b):
    from concourse.bass_utils import run_bass_kernel_spmd

    nc = _get(S, CHUNK)
    in_maps = _make_in_maps(x, h0, w_h_w, w_h_b, w_z_w, w_z_b)
    res = run_bass_kernel_spmd(nc, in_maps, list(range(N_CORES)))
    out = np.empty((N_CORES, S, H), dtype=np.float32)
    for i in range(N_CORES):
        hT = np.asarray(res.results[i]["out"]).reshape(H, S)
        out[i] = hT.astype(np.float32).T
    return out
